# revision 1
# baseline (speedup 1.0000x reference)
"""Dense-CRF mean-field inference on 8 Trainium2 NeuronCores.

Math restructuring (validated numerically against the jax reference):
  - Kb + Kg share weight 1.0 -> single kernel matrix K = exp(-.5 d2_b) + exp(-.5 d2_g).
  - The Potts 3x3 conv update is  upd[c] = boxsum3(S) - boxsum3(comb[c]) with
    S = sum_c comb[c]; the S part is class-independent so softmax drops it:
        out = softmax(input + UPDATE_FACTOR * boxsum3(comb[c])).
    The UPDATE_FACTOR (3.0) is folded into K via exp(x + ln 3).
  - Spatial sigma 5 -> K decays fast with |dy|; rows further than ~20 image rows
    from the output pixel contribute < 1e-5 relative.  Each core keeps a
    41-block (5248 px) band of K rows resident in SBUF: blocks within +-6 rows
    in fp32, the rest fp16 (validated: l2 rel err 2.9e-5 vs fp32-exact 2.2e-5).
  - -0.5*||fi-fj||^2 is computed by ONE matmul per kernel via augmented
    features: G=[y,x,-.5|s|^2,1,r,g,b,-.5|c|^2,1], H=[y,x,1,-.5|s|^2,r,g,b,1,-.5|c|^2];
    gaussian = rows 0:4, bilateral = rows 0:9.
  - Each core computes comb for 14 image rows (its 12 + 1 halo row each side,
    edge rows duplicated via clamped features) so the 3x3 conv is local.
    One AllGather of the new per-core probabilities per iteration.

Sharding: core r owns output image rows [12r, 12r+12); K band = global
128-px blocks [9r-16, 9r+25) (zero-K padding outside the image).
"""

import os
import sys

import numpy as np

for _p in ("/opt/trn_rl_repo",):
    if _p not in sys.path and os.path.isdir(_p):
        sys.path.insert(0, _p)

H = 96
W = 96
C = 5
N = H * W                      # 9216
NCORES = 8
RPC = H // NCORES              # 12 image rows per core
NLOC = (RPC + 2) * W           # 1344 extended-output pixels (14 rows)
NMID = RPC * W                 # 1152 owned pixels
BLK = 128
NBLK = 41                      # K band m-blocks per core
BAND_LO = -16                  # band start, in global blocks, relative to 9r
F32_LO, F32_HI = 12, 29        # band-local block range kept in fp32 (+-4 rows)
N32 = F32_HI - F32_LO          # 21 fp32 blocks
N16 = NBLK - N32               # 20 fp16 blocks
GBLK = N // BLK                # 72 global blocks
PADBLK = 16                    # padding blocks each side of flat_padded
FPW = (GBLK + 2 * PADBLK) * C  # flat_padded free width = 520
CH = 448                       # matvec/exp n-chunk (fits one PSUM bank)
NCH = 3
ITERS = 5
LN3 = float(np.log(3.0))
NEG = -1.0e30                  # kills exp() for out-of-image padding blocks

_CACHED_NC = None


def _near(i):
    return F32_LO <= i < F32_HI


def _k16_idx(i):
    return i if i < F32_LO else i - N32


def _build_module():
    import concourse.bass as bass
    import concourse.bacc as bacc
    import concourse.tile as tile
    from concourse import mybir
    from concourse.masks import make_identity

    f32 = mybir.dt.float32
    f16 = mybir.dt.float16
    u32 = mybir.dt.uint32
    EXP = mybir.ActivationFunctionType.Exp
    COPY = mybir.ActivationFunctionType.Copy

    nc = bacc.Bacc("TRN2", target_bir_lowering=False, debug=False,
                   num_devices=NCORES)

    g_dram = nc.dram_tensor("g_feats", [9, NBLK * BLK], f32, kind="ExternalInput")
    h_dram = nc.dram_tensor("h_feats", [9, NLOC], f32, kind="ExternalInput")
    ipp_dram = nc.dram_tensor("inp_pp", [BLK, GBLK * C], f32, kind="ExternalInput")
    icn_dram = nc.dram_tensor("inp_cn", [C, NMID], f32, kind="ExternalInput")
    boff_dram = nc.dram_tensor("band_off", [1, 1], u32, kind="ExternalInput")
    kg32_dram = nc.dram_tensor("kg32", [BLK, N32 * NCH * CH], f32,
                               kind="ExternalInput")
    kg16_dram = nc.dram_tensor("kg16", [BLK, N16 * NCH * CH], f16,
                               kind="ExternalInput")
    out_dram = nc.dram_tensor("out_loc", [BLK, (NMID // BLK) * C], f32,
                              kind="ExternalOutput")

    def bcast_inner(ap, n):
        return bass.AP(tensor=ap.tensor, offset=ap.offset, ap=[*ap.ap, [0, n]])

    with tile.TileContext(nc) as tc:
        with tc.tile_pool(name="singles", bufs=1) as singles, \
             tc.tile_pool(name="warmps", bufs=1, space="PSUM") as warmpool, \
             tc.tile_pool(name="dram", bufs=1, space="DRAM") as dram:

            # ---- long-lived SBUF state ----
            k32 = singles.tile([BLK, N32, NCH * CH], f32, name="k32")
            k16 = singles.tile([BLK, N16, NCH * CH], f16, name="k16")
            flat_pad = singles.tile([BLK, FPW], f32, name="flat_pad")
            h_sb = singles.tile([9, NLOC], f32, name="h_sb")
            ipp_sb = singles.tile([BLK, GBLK * C], f32, name="ipp_sb")
            icn_sb = singles.tile([C, NMID], f32, name="icn_sb")
            ident = singles.tile([BLK, BLK], f32, name="ident")
            boff_sb = singles.tile([1, 1], u32, name="boff_sb")
            ln3_sb = singles.tile([BLK, 1], f32, name="ln3_sb")
            nc.vector.memset(ln3_sb, LN3)
            # HAM warm-keeper: dummy matmuls that fill PE-idle windows so the
            # activity monitor keeps the PE clock at 2.4 GHz (it halves the
            # clock after ~3.4us of idle).  ~426 ns each (fp32 512-col).
            warm_ps = warmpool.tile([1, 512], f32, name="warm_ps")

            def warm(n):
                for _ in range(n):
                    nc.tensor.matmul(warm_ps, ident[:, 0:1], k32[:, 0, 0:512],
                                     start=True, stop=True)

            ag_in = dram.tile([BLK, (NMID // BLK) * C], f32, name="ag_in")
            ag_out = dram.tile([BLK * NCORES, (NMID // BLK) * C], f32, name="ag_out")

            nc.sync.dma_start(out=h_sb, in_=h_dram[:, :])
            nc.sync.dma_start(out=ipp_sb, in_=ipp_dram[:, :])
            nc.sync.dma_start(out=icn_sb, in_=icn_dram[:, :])
            nc.sync.dma_start(out=boff_sb, in_=boff_dram[:, :])
            make_identity(nc, ident)
            nc.vector.memset(flat_pad, 0.0)

            # band offset register (elements into flat_pad) = 45 * core_id
            boff_regs = nc.alloc_registers("boff_regs",
                                           engines=(mybir.EngineType.DVE,))
            nc.regs_load(boff_regs, boff_sb[0:1, 0:1])
            off_sv = nc.snap(boff_regs, donate=True, min_val=0,
                             max_val=(NCORES - 1) * 9 * C)

            # ---- phase 1: build K band ----
            # Bilateral part on device (input-dependent); the gaussian part is
            # input-independent so the host ships it precomputed (kg32/kg16)
            # and we just add it.
            with tc.tile_pool(name="gstage", bufs=3) as gpool, \
                 tc.tile_pool(name="kgstage", bufs=3) as kgpool, \
                 tc.tile_pool(name="bpsum", bufs=2, space="PSUM") as bppool:
                for i in range(NBLK):
                    gt = gpool.tile([9, BLK], f32, tag="gt")
                    nc.sync.dma_start(out=gt, in_=g_dram[:, i * BLK:(i + 1) * BLK])
                    if _near(i):
                        kdst = k32[:, i - F32_LO, :]
                        kdt = f32
                        j = i - F32_LO
                        kg_src = kg32_dram[:, j * NCH * CH:(j + 1) * NCH * CH]
                    else:
                        kdst = k16[:, _k16_idx(i), :]
                        kdt = f16
                        j = _k16_idx(i)
                        kg_src = kg16_dram[:, j * NCH * CH:(j + 1) * NCH * CH]
                    kg = kgpool.tile([BLK, NCH * CH], kdt, tag="kg")
                    nc.sync.dma_start(out=kg, in_=kg_src)
                    pb = bppool.tile([BLK, NCH, 512], f32, tag="pb")
                    for nb in range(NCH):
                        hs = h_sb[:, nb * CH:(nb + 1) * CH]
                        nc.tensor.matmul(pb[:, nb, 0:CH], gt[0:9, :], hs[0:9, :],
                                         start=True, stop=True)
                    kv = kdst.rearrange("p (a c) -> p a c", c=CH)
                    nc.scalar.activation(out=kv, in_=pb[:, :, 0:CH], func=EXP,
                                         bias=ln3_sb)
                    nc.vector.tensor_add(kdst, kdst, kg)
                warm(12)

            # ---- helpers ----
            def softmax_pp(pool, u_pp, mb, tag):
                """u_pp: [128, mb*C] logits, pixel-partition layout -> probs."""
                v = u_pp.rearrange("p (a c) -> p a c", c=C)
                mx = pool.tile([BLK, mb], f32, tag=f"{tag}_mx")
                nc.vector.tensor_reduce(out=mx, in_=v,
                                        axis=mybir.AxisListType.X,
                                        op=mybir.AluOpType.max)
                e = pool.tile([BLK, mb * C], f32, tag=f"{tag}_e")
                ev = e.rearrange("p (a c) -> p a c", c=C)
                nc.vector.tensor_sub(ev, v, bcast_inner(mx, C))
                nc.scalar.activation(out=e, in_=e, func=EXP)
                s = pool.tile([BLK, mb], f32, tag=f"{tag}_s")
                nc.vector.tensor_reduce(out=s, in_=ev,
                                        axis=mybir.AxisListType.X,
                                        op=mybir.AluOpType.add)
                nc.vector.reciprocal(out=s, in_=s)
                fl = pool.tile([BLK, mb * C], f32, tag=f"{tag}_fl")
                nc.vector.tensor_mul(fl.rearrange("p (a c) -> p a c", c=C), ev,
                                     bcast_inner(s, C))
                return fl

            # ---- phase 2: initial flat = softmax(input) ----
            with tc.tile_pool(name="init", bufs=1) as ipool:
                fl0 = softmax_pp(ipool, ipp_sb, GBLK, "sm0")
                nc.vector.tensor_copy(
                    out=flat_pad[:, PADBLK * C:(PADBLK + GBLK) * C], in_=fl0)

            # ---- phase 3: iterations ----
            with tc.tile_pool(name="iter", bufs=1) as wpool, \
                 tc.tile_pool(name="band", bufs=2) as bpool, \
                 tc.tile_pool(name="smx", bufs=2) as spool, \
                 tc.tile_pool(name="ipsum", bufs=2, space="PSUM") as ippool:
                for it in range(ITERS):
                    band32 = bpool.tile([BLK, NBLK * C], f32, tag="band32")
                    nc.vector.tensor_copy(
                        out=band32, in_=flat_pad[:, bass.ds(off_sv, NBLK * C)])
                    band16 = bpool.tile([BLK, NBLK * C], f16, tag="band16")
                    nc.vector.tensor_copy(out=band16, in_=band32)

                    # matvec: comb[c, n] = sum_m K[m, n] * flat[c, m]
                    pv = ippool.tile([C, NCH, 512], f32, tag="pv", bufs=1)
                    for nb in range(NCH):
                        for i in range(NBLK):
                            if _near(i):
                                lhs = band32[:, i * C:(i + 1) * C]
                                kt = k32[:, i - F32_LO, nb * CH:(nb + 1) * CH]
                            else:
                                lhs = band16[:, i * C:(i + 1) * C]
                                kt = k16[:, _k16_idx(i), nb * CH:(nb + 1) * CH]
                            nc.tensor.matmul(pv[:, nb, 0:CH], lhs, kt,
                                             start=(i == 0), stop=(i == NBLK - 1))
                    warm(20)
                    comb = wpool.tile([C, NLOC], f32, tag="comb")
                    nc.scalar.activation(
                        out=comb.rearrange("p (a c) -> p a c", c=CH),
                        in_=pv[:, :, 0:CH], func=COPY)

                    # 3x3 box sum: x-pass into t1 (all 14 rows), edge-replicated
                    t1 = wpool.tile([C, NLOC], f32, tag="t1")
                    nc.vector.tensor_add(t1[:, 1:NLOC - 1], comb[:, 0:NLOC - 2],
                                         comb[:, 2:NLOC])
                    nc.vector.tensor_add(t1[:, 1:NLOC - 1], t1[:, 1:NLOC - 1],
                                         comb[:, 1:NLOC - 1])
                    t1r = t1.rearrange("p (row x) -> p row x", x=W)
                    cbr = comb.rearrange("p (row x) -> p row x", x=W)
                    # x = 0 column: 2*c[0] + c[1]
                    nc.vector.tensor_add(t1r[:, :, 0:1], cbr[:, :, 0:1],
                                         cbr[:, :, 1:2])
                    nc.vector.tensor_add(t1r[:, :, 0:1], t1r[:, :, 0:1],
                                         cbr[:, :, 0:1])
                    # x = W-1 column: c[W-2] + 2*c[W-1]
                    nc.vector.tensor_add(t1r[:, :, W - 1:W], cbr[:, :, W - 2:W - 1],
                                         cbr[:, :, W - 1:W])
                    nc.vector.tensor_add(t1r[:, :, W - 1:W], t1r[:, :, W - 1:W],
                                         cbr[:, :, W - 1:W])
                    # y-pass (middle 12 rows) + input logits
                    u = wpool.tile([C, NMID], f32, tag="u")
                    nc.vector.tensor_add(u, t1[:, 0:NMID], t1[:, 2 * W:NLOC])
                    nc.vector.tensor_add(u, u, t1[:, W:NMID + W])
                    nc.vector.tensor_add(u, u, icn_sb)

                    # transpose U [5, 1152] -> pixel-partition [128, 9*5]
                    u_pp = spool.tile([BLK, (NMID // BLK) * C], f32, tag="u_pp")
                    for kb in range(NMID // BLK):
                        pt = ippool.tile([BLK, C], f32, tag="pt")
                        nc.tensor.transpose(pt, u[:, kb * BLK:(kb + 1) * BLK],
                                            ident[0:C, 0:C])
                        nc.vector.tensor_copy(out=u_pp[:, kb * C:(kb + 1) * C],
                                              in_=pt)

                    flat_l = softmax_pp(spool, u_pp, NMID // BLK, "smx")
                    if it < ITERS - 1:
                        warm(42)

                    if it < ITERS - 1:
                        nc.sync.dma_start(out=ag_in, in_=flat_l)
                        nc.gpsimd.collective_compute(
                            "AllGather",
                            mybir.AluOpType.bypass,
                            replica_groups=[list(range(NCORES))],
                            ins=[ag_in.opt()],
                            outs=[ag_out.opt()],
                        )
                        nc.sync.dma_start(
                            out=flat_pad[:, PADBLK * C:(PADBLK + GBLK) * C]
                            .rearrange("p (r j) -> p r j", r=NCORES),
                            in_=ag_out.rearrange("(r p) j -> p r j", p=BLK))
                    else:
                        nc.sync.dma_start(out=out_dram[:, :], in_=flat_l)

    nc.compile()
    return nc


def _host_inputs(input_tensor, reference_tensor):
    logits = np.ascontiguousarray(
        np.asarray(input_tensor, dtype=np.float32)[0].reshape(C, N))
    ref = np.asarray(reference_tensor, dtype=np.float32)[0]  # [3, 96, 96]

    yy, xx = np.meshgrid(np.arange(H, dtype=np.float32),
                         np.arange(W, dtype=np.float32), indexing="ij")
    Y = (yy / 5.0).reshape(N)
    X = (xx / 5.0).reshape(N)
    RGB = (ref / 0.5).reshape(3, N)
    s2 = -0.5 * (Y * Y + X * X)
    c2 = -0.5 * (RGB * RGB).sum(axis=0)
    ones = np.ones(N, np.float32)

    # G (band / m side) and H (output / n side) augmented features
    G_all = np.stack([Y, X, s2, ones, RGB[0], RGB[1], RGB[2], c2, ones])
    H_all = np.stack([Y, X, ones, s2, RGB[0], RGB[1], RGB[2], ones, c2])

    # input in pixel-partition layout [128, 72*5]
    ipp = np.ascontiguousarray(
        logits.reshape(C, GBLK, BLK).transpose(2, 1, 0).reshape(BLK, GBLK * C))

    # gaussian kernel tables: 3*exp(-(dy^2+dx^2)/50), folded update factor 3
    dtab = np.exp(-(np.arange(-(H - 1), H) ** 2) / 50.0).astype(np.float64)
    gx3 = (3.0 * dtab).astype(np.float32)
    gy1 = dtab.astype(np.float32)
    yy_all = (np.arange(N) // W).astype(np.int64)
    xx_all = (np.arange(N) % W).astype(np.int64)

    def kg_for_core(r, yn, xn):
        """[NBLK, 128, 1344] gaussian kernel values for core r's band."""
        kg = np.zeros((NBLK, BLK, NLOC), np.float32)
        for i in range(NBLK):
            gb = 9 * r + BAND_LO + i
            if 0 <= gb < GBLK:
                pm = np.arange(gb * BLK, (gb + 1) * BLK)
                A = gy1[yy_all[pm][:, None] - yn[None, :] + H - 1]
                B = gx3[xx_all[pm][:, None] - xn[None, :] + H - 1]
                kg[i] = A * B
        return kg

    in_maps = []
    kg_interior = None
    for r in range(NCORES):
        g = np.zeros((9, NBLK * BLK), np.float32)
        g[2, :] = NEG
        for i in range(NBLK):
            gb = 9 * r + BAND_LO + i
            if 0 <= gb < GBLK:
                g[:, i * BLK:(i + 1) * BLK] = G_all[:, gb * BLK:(gb + 1) * BLK]
        yext = np.clip(np.arange(RPC * r - 1, RPC * (r + 1) + 1), 0, H - 1)
        hpix = (yext[:, None] * W + np.arange(W)[None, :]).reshape(-1)
        h = np.ascontiguousarray(H_all[:, hpix])
        icn = np.ascontiguousarray(
            logits.reshape(C, H, W)[:, RPC * r:RPC * (r + 1), :].reshape(C, NMID))
        # gaussian part of K (interior cores share one array)
        if 2 <= r <= 5:
            if kg_interior is None:
                kg_interior = kg_for_core(r, yy_all[hpix], xx_all[hpix])
            kg = kg_interior
        else:
            kg = kg_for_core(r, yy_all[hpix], xx_all[hpix])
        near_idx = list(range(F32_LO, F32_HI))
        far_idx = [i for i in range(NBLK) if not _near(i)]
        far_idx = sorted(far_idx, key=_k16_idx)
        kg32 = np.ascontiguousarray(
            kg[near_idx].transpose(1, 0, 2).reshape(BLK, N32 * NLOC))
        kg16 = np.ascontiguousarray(
            kg[far_idx].transpose(1, 0, 2).reshape(BLK, N16 * NLOC)
        ).astype(np.float16)
        in_maps.append({
            "g_feats": g,
            "h_feats": h,
            "inp_pp": ipp,
            "inp_cn": icn,
            "band_off": np.array([[9 * C * r]], np.uint32),
            "kg32": kg32,
            "kg16": kg16,
        })
    return in_maps


def _assemble(results):
    out = np.empty((C, N), np.float32)
    for r in range(NCORES):
        blk = results[r]["out_loc"].reshape(BLK, NMID // BLK, C)
        out[:, NMID * r:NMID * (r + 1)] = (
            blk.transpose(2, 1, 0).reshape(C, NMID))
    return out.reshape(1, C, H, W)


def _get_nc():
    global _CACHED_NC
    if _CACHED_NC is None:
        _CACHED_NC = _build_module()
    return _CACHED_NC


def run(input_tensor, reference_tensor, trace=False):
    from concourse.bass_utils import run_bass_kernel_spmd
    nc = _get_nc()
    in_maps = _host_inputs(input_tensor, reference_tensor)
    res = run_bass_kernel_spmd(nc, in_maps, core_ids=list(range(NCORES)),
                               trace=trace)
    return _assemble(res.results), res


def kernel(input_tensor, reference_tensor):
    out, _ = run(input_tensor, reference_tensor, trace=False)
    return out



# revision 5
# speedup vs baseline: 1.8611x; 1.8611x over previous
"""Dense-CRF mean-field inference on 8 Trainium2 NeuronCores.

Math restructuring (validated numerically against the jax reference):
  - Kb and Kg share the spatial sigma (5.0), so
        K = Kb + Kg = Kg * (1 + Cc),
    where Cc = exp(-.5||ci-cj||^2/sig_c^2) is a pure COLOR Gaussian.
    Only Cc is input-dependent; Kg (and the x3 UPDATE_FACTOR fold) is
    separable spatial structure the host precomputes as per-block
    rank-1 factors gy[128,14] (x) gx[128,96].
  - Color feature products are <= ~6 in magnitude -> the Cc feature
    matmul is fp16-safe; the whole K band lives in SBUF as fp16
    (1 PE cycle/row vs 4 for fp32). Simulated end-to-end rel err 5e-4
    vs the 2e-2 gate.
  - The Potts 3x3 conv update reduces to out = softmax(input +
    boxsum3(comb)) (class-independent part drops in softmax).
  - Band: 37 global 128-px blocks per core (sim: 37 -> 5e-4, 33 ->
    2.6e-2, so 37 is the minimum safe width). Per-core band order is
    [own 9 | left 14 | right 14] so runtime ds() offsets can split the
    flat copy; out-of-image blocks get gy=0 -> K=0.
  - One fp16 AllGather of the per-core probabilities per iteration.

Sharding: core r owns output image rows [12r, 12r+12).
"""

import os
import sys

import numpy as np

for _p in ("/opt/trn_rl_repo",):
    if _p not in sys.path and os.path.isdir(_p):
        sys.path.insert(0, _p)

H = 96
W = 96
C = 5
N = H * W                      # 9216
NCORES = 8
RPC = H // NCORES              # 12 image rows per core
EXT = RPC + 2                  # 14 rows incl. 1 halo row each side
NLOC = EXT * W                 # 1344 extended-output pixels
NMID = RPC * W                 # 1152 owned pixels
BLK = 128
NBLK = 37                      # K band m-blocks per core
HB = (NBLK - 9) // 2           # 14 blocks each side of the 9 own
GBLK = N // BLK                # 72 global blocks
PADBLK = HB                    # padding blocks each side of flat_pad
FPW = (GBLK + 2 * PADBLK) * C  # flat_pad free width = 500
# matvec n-chunks, row-aligned so the x-box can read PSUM directly
CHROWS = (5, 5, 4)
CHS = [r * W for r in CHROWS]  # (480, 480, 384)
CH0 = [sum(CHS[:j]) for j in range(3)]
BCH = 448                      # build n-chunk (fits one PSUM bank)
ITERS = 5

_CACHED_NC = None


def _build_module():
    import concourse.bass as bass
    import concourse.bacc as bacc
    import concourse.tile as tile
    from concourse import mybir
    from concourse.masks import make_identity

    f32 = mybir.dt.float32
    f16 = mybir.dt.float16
    u32 = mybir.dt.uint32
    EXP = mybir.ActivationFunctionType.Exp
    COPY = mybir.ActivationFunctionType.Copy
    ADD = mybir.AluOpType.add
    MULT = mybir.AluOpType.mult

    nc = bacc.Bacc("TRN2", target_bir_lowering=False, debug=False,
                   num_devices=NCORES)

    g_dram = nc.dram_tensor("g_feats", [C, NBLK * BLK], f16, kind="ExternalInput")
    h_dram = nc.dram_tensor("h_feats", [C, NLOC], f16, kind="ExternalInput")
    gy_dram = nc.dram_tensor("gy_fac", [BLK, NBLK * EXT], f16, kind="ExternalInput")
    gx_dram = nc.dram_tensor("gx_fac", [BLK, NBLK * W], f16, kind="ExternalInput")
    ipp_dram = nc.dram_tensor("inp_pp", [BLK, GBLK * C], f32, kind="ExternalInput")
    icn_dram = nc.dram_tensor("inp_cn", [C, NMID], f32, kind="ExternalInput")
    off_dram = nc.dram_tensor("offsets", [1, 3], u32, kind="ExternalInput")
    out_dram = nc.dram_tensor("out_loc", [BLK, (NMID // BLK) * C], f32,
                              kind="ExternalOutput")

    def bcast_inner(ap, n):
        return bass.AP(tensor=ap.tensor, offset=ap.offset, ap=[*ap.ap, [0, n]])

    def bcast_mid(ap, n):
        # [p, q] -> [p, n, q] with stride-0 middle dim
        return bass.AP(tensor=ap.tensor, offset=ap.offset,
                       ap=[ap.ap[0], [0, n], *ap.ap[1:]])

    with tile.TileContext(nc) as tc:
        with tc.tile_pool(name="singles", bufs=1) as singles, \
             tc.tile_pool(name="warmps", bufs=1, space="PSUM") as warmpool, \
             tc.tile_pool(name="dram", bufs=1, space="DRAM") as dram:

            # ---- long-lived SBUF state ----
            k16 = singles.tile([BLK, NBLK, NLOC], f16, name="k16")
            flat_pad = singles.tile([BLK, FPW], f16, name="flat_pad")
            g_sb = singles.tile([C, NBLK * BLK], f16, name="g_sb")
            h_sb = singles.tile([C, NLOC], f16, name="h_sb")
            gy_sb = singles.tile([BLK, NBLK * EXT], f16, name="gy_sb")
            gx_sb = singles.tile([BLK, NBLK * W], f16, name="gx_sb")
            ipp_sb = singles.tile([BLK, GBLK * C], f32, name="ipp_sb")
            icn_sb = singles.tile([C, NMID], f32, name="icn_sb")
            ident = singles.tile([BLK, BLK], f32, name="ident")
            off_sb = singles.tile([1, 3], u32, name="off_sb")
            warm_ps = warmpool.tile([1, 512], f32, name="warm_ps")

            nc.sync.dma_start(out=g_sb, in_=g_dram[:, :])
            nc.sync.dma_start(out=h_sb, in_=h_dram[:, :])
            nc.sync.dma_start(out=gy_sb, in_=gy_dram[:, :])
            nc.sync.dma_start(out=gx_sb, in_=gx_dram[:, :])
            nc.sync.dma_start(out=ipp_sb, in_=ipp_dram[:, :])
            nc.sync.dma_start(out=icn_sb, in_=icn_dram[:, :])
            nc.sync.dma_start(out=off_sb, in_=off_dram[:, :])
            make_identity(nc, ident)
            nc.vector.memset(flat_pad, 0.0)

            # runtime flat_pad element offsets: own / left / right windows
            offs = []
            for j, mx in enumerate(((PADBLK + 9 * (NCORES - 1)) * C,
                                    (PADBLK + 9 * (NCORES - 1) - HB) * C,
                                    (PADBLK + 9 * (NCORES - 1) + 9) * C)):
                regs = nc.alloc_registers(f"off_regs{j}",
                                          engines=(mybir.EngineType.DVE,))
                nc.regs_load(regs, off_sb[0:1, j:j + 1])
                offs.append(nc.snap(regs, donate=True, min_val=0, max_val=mx))
            own_off, left_off, right_off = offs

            # HAM warm-keeper: fp16 matmuls (~213 ns each) that fill PE-idle
            # windows so the activity monitor keeps the PE clock at 2.4 GHz.
            def warm(n):
                for _ in range(n):
                    nc.tensor.matmul(warm_ps, k16[:, 0, 0:1], k16[:, 0, 0:512],
                                     start=True, stop=True)

            ag_in = dram.tile([BLK, (NMID // BLK) * C], f16, name="ag_in")
            ag_out = dram.tile([BLK * NCORES, (NMID // BLK) * C], f16,
                               name="ag_out")

            # ---- phase 1: build the fp16 K band ----
            # per block i: ccarg = G_i^T H (5-deep fp16 matmul) ->
            # t = exp(ccarg) into k16[i] -> k16[i] = (t + 1) * (gy (x) gx)
            with tc.tile_pool(name="kgstage", bufs=3) as kgpool, \
                 tc.tile_pool(name="bpsum", bufs=2, space="PSUM") as bppool:
                for i in range(NBLK):
                    gt = g_sb[:, i * BLK:(i + 1) * BLK]
                    pb = bppool.tile([BLK, 3, 512], f32, tag="pb")
                    for nb in range(3):
                        hs = h_sb[:, nb * BCH:(nb + 1) * BCH]
                        nc.tensor.matmul(pb[:, nb, 0:BCH], gt, hs,
                                         start=True, stop=True)
                    ksl = k16[:, i, :]
                    nc.scalar.activation(
                        out=ksl.rearrange("p (a c) -> p a c", c=BCH),
                        in_=pb[:, :, 0:BCH], func=EXP)
                    kg = kgpool.tile([BLK, NLOC], f16, tag="kg")
                    gyb = gy_sb[:, i * EXT:(i + 1) * EXT]
                    gxb = gx_sb[:, i * W:(i + 1) * W]
                    nc.vector.tensor_mul(
                        kg.rearrange("p (r x) -> p r x", x=W),
                        bcast_inner(gyb, W), bcast_mid(gxb, EXT))
                    nc.vector.scalar_tensor_tensor(
                        out=ksl, in0=ksl, scalar=1.0, in1=kg,
                        op0=ADD, op1=MULT)

            # ---- helpers ----
            def softmax_pp(pool, u_pp, mb, tag, out_dt=f32):
                """u_pp: [128, mb*C] logits, pixel-partition layout -> probs."""
                v = u_pp.rearrange("p (a c) -> p a c", c=C)
                mx = pool.tile([BLK, mb], f32, tag=f"{tag}_mx")
                nc.vector.tensor_reduce(out=mx, in_=v,
                                        axis=mybir.AxisListType.X,
                                        op=mybir.AluOpType.max)
                e = pool.tile([BLK, mb * C], f32, tag=f"{tag}_e")
                ev = e.rearrange("p (a c) -> p a c", c=C)
                nc.vector.tensor_sub(ev, v, bcast_inner(mx, C))
                nc.scalar.activation(out=e, in_=e, func=EXP)
                s = pool.tile([BLK, mb], f32, tag=f"{tag}_s")
                nc.vector.tensor_reduce(out=s, in_=ev,
                                        axis=mybir.AxisListType.X,
                                        op=mybir.AluOpType.add)
                nc.vector.reciprocal(out=s, in_=s)
                fl = pool.tile([BLK, mb * C], out_dt, tag=f"{tag}_fl")
                nc.vector.tensor_mul(fl.rearrange("p (a c) -> p a c", c=C), ev,
                                     bcast_inner(s, C))
                return fl

            # ---- phase 2: initial flat = softmax(input), all cores alike ----
            with tc.tile_pool(name="init", bufs=1) as ipool:
                fl0 = softmax_pp(ipool, ipp_sb, GBLK, "sm0", out_dt=f16)
                nc.vector.tensor_copy(
                    out=flat_pad[:, PADBLK * C:(PADBLK + GBLK) * C], in_=fl0)

            # ---- phase 3: iterations ----
            with tc.tile_pool(name="iter", bufs=1) as wpool, \
                 tc.tile_pool(name="band", bufs=2) as bpool, \
                 tc.tile_pool(name="smx", bufs=2) as spool, \
                 tc.tile_pool(name="ipsum", bufs=2, space="PSUM") as ippool:
                for it in range(ITERS):
                    band = bpool.tile([BLK, NBLK * C], f16, tag="band")
                    nc.vector.tensor_copy(
                        out=band[:, 0:9 * C],
                        in_=flat_pad[:, bass.ds(own_off, 9 * C)])
                    nc.vector.tensor_copy(
                        out=band[:, 9 * C:(9 + HB) * C],
                        in_=flat_pad[:, bass.ds(left_off, HB * C)])
                    nc.vector.tensor_copy(
                        out=band[:, (9 + HB) * C:NBLK * C],
                        in_=flat_pad[:, bass.ds(right_off, HB * C)])

                    # matvec: comb[c, n] = sum_m K[m, n] * flat[c, m]
                    pv = ippool.tile([C, 3, 512], f32, tag="pv", bufs=1)
                    for nb in range(3):
                        for i in range(NBLK):
                            nc.tensor.matmul(
                                pv[:, nb, 0:CHS[nb]],
                                band[:, i * C:(i + 1) * C],
                                k16[:, i, CH0[nb]:CH0[nb] + CHS[nb]],
                                start=(i == 0), stop=(i == NBLK - 1))
                    warm(10)

                    # 3x3 box sum, x-pass per row-aligned PSUM chunk
                    # (hardware: at most one PSUM operand per vector op, so
                    # seed t1 with a scalar-engine copy then accumulate)
                    t1 = wpool.tile([C, NLOC], f32, tag="t1")
                    t1r = t1.rearrange("p (row x) -> p row x", x=W)
                    r0 = 0
                    for nb in range(3):
                        nr = CHROWS[nb]
                        cb = pv[:, nb, 0:CHS[nb]].rearrange(
                            "p (row x) -> p row x", x=W)
                        tb = t1r[:, r0:r0 + nr, :]
                        nc.scalar.activation(out=tb, in_=cb, func=COPY)
                        nc.vector.tensor_add(tb[:, :, 1:W - 1],
                                             tb[:, :, 1:W - 1],
                                             cb[:, :, 0:W - 2])
                        nc.vector.tensor_add(tb[:, :, 1:W - 1],
                                             tb[:, :, 1:W - 1],
                                             cb[:, :, 2:W])
                        nc.vector.tensor_add(tb[:, :, 0:1], tb[:, :, 0:1],
                                             cb[:, :, 0:1])
                        nc.vector.tensor_add(tb[:, :, 0:1], tb[:, :, 0:1],
                                             cb[:, :, 1:2])
                        nc.vector.tensor_add(tb[:, :, W - 1:W],
                                             tb[:, :, W - 1:W],
                                             cb[:, :, W - 1:W])
                        nc.vector.tensor_add(tb[:, :, W - 1:W],
                                             tb[:, :, W - 1:W],
                                             cb[:, :, W - 2:W - 1])
                        r0 += nr
                    # y-pass (middle 12 rows) + input logits
                    u = wpool.tile([C, NMID], f32, tag="u")
                    nc.vector.tensor_add(u, t1[:, 0:NMID], t1[:, 2 * W:NLOC])
                    nc.vector.tensor_add(u, u, t1[:, W:NMID + W])
                    nc.vector.tensor_add(u, u, icn_sb)

                    # transpose U [5, 1152] -> pixel-partition psum [128, 9*5]
                    pt = ippool.tile([BLK, (NMID // BLK) * C], f32, tag="pt")
                    for kb in range(NMID // BLK):
                        nc.tensor.transpose(pt[:, kb * C:(kb + 1) * C],
                                            u[:, kb * BLK:(kb + 1) * BLK],
                                            ident[0:C, 0:C])

                    if it < ITERS - 1:
                        flat_l = softmax_pp(spool, pt, NMID // BLK, "smx",
                                            out_dt=f16)
                        nc.sync.dma_start(out=ag_in, in_=flat_l)
                        nc.gpsimd.collective_compute(
                            "AllGather",
                            mybir.AluOpType.bypass,
                            replica_groups=[list(range(NCORES))],
                            ins=[ag_in.opt()],
                            outs=[ag_out.opt()],
                        )
                        warm(44)
                        nc.sync.dma_start(
                            out=flat_pad[:, PADBLK * C:(PADBLK + GBLK) * C]
                            .rearrange("p (r j) -> p r j", r=NCORES),
                            in_=ag_out.rearrange("(r p) j -> p r j", p=BLK))
                    else:
                        flat_l = softmax_pp(spool, pt, NMID // BLK, "smx",
                                            out_dt=f32)
                        nc.sync.dma_start(out=out_dram[:, :], in_=flat_l)

    nc.compile()
    return nc


def _host_inputs(input_tensor, reference_tensor):
    logits = np.ascontiguousarray(
        np.asarray(input_tensor, dtype=np.float32)[0].reshape(C, N))
    ref = np.asarray(reference_tensor, dtype=np.float32)[0]  # [3, 96, 96]

    RGB = (ref / 0.5).reshape(3, N).astype(np.float32)
    c2 = (-0.5 * (RGB * RGB).sum(axis=0)).astype(np.float32)
    ones = np.ones(N, np.float32)
    G_all = np.stack([RGB[0], RGB[1], RGB[2], c2, ones]).astype(np.float16)
    H_all = np.stack([RGB[0], RGB[1], RGB[2], ones, c2]).astype(np.float16)

    # input in pixel-partition layout [128, 72*5]
    ipp = np.ascontiguousarray(
        logits.reshape(C, GBLK, BLK).transpose(2, 1, 0).reshape(BLK, GBLK * C))

    # spatial gaussian tables; x table carries the 3.0 UPDATE_FACTOR fold
    dtab = np.exp(-(np.arange(-(H - 1), H) ** 2) / 50.0)
    gy1 = dtab.astype(np.float16)
    gx3 = (3.0 * dtab).astype(np.float16)
    yy_all = (np.arange(N) // W).astype(np.int64)
    xx_all = (np.arange(N) % W).astype(np.int64)

    in_maps = []
    for r in range(NCORES):
        # band-local order: [own 9 | left 14 | right 14] global blocks
        order = (list(range(9 * r, 9 * r + 9))
                 + list(range(9 * r - HB, 9 * r))
                 + list(range(9 * r + 9, 9 * r + 9 + HB)))
        yext = np.clip(np.arange(RPC * r - 1, RPC * (r + 1) + 1), 0, H - 1)
        g = np.zeros((C, NBLK * BLK), np.float16)
        gy = np.zeros((BLK, NBLK * EXT), np.float16)
        gx = np.zeros((BLK, NBLK * W), np.float16)
        for i, gb in enumerate(order):
            if 0 <= gb < GBLK:
                pm = np.arange(gb * BLK, (gb + 1) * BLK)
                g[:, i * BLK:(i + 1) * BLK] = G_all[:, pm]
                gy[:, i * EXT:(i + 1) * EXT] = gy1[
                    yy_all[pm][:, None] - yext[None, :] + H - 1]
                gx[:, i * W:(i + 1) * W] = gx3[
                    xx_all[pm][:, None] - np.arange(W)[None, :] + H - 1]
        hpix = (yext[:, None] * W + np.arange(W)[None, :]).reshape(-1)
        h = np.ascontiguousarray(H_all[:, hpix])
        icn = np.ascontiguousarray(
            logits.reshape(C, H, W)[:, RPC * r:RPC * (r + 1), :].reshape(C, NMID))
        offsets = np.array([[(PADBLK + 9 * r) * C,
                             9 * r * C,
                             (PADBLK + 9 * r + 9) * C]], np.uint32)
        in_maps.append({
            "g_feats": g,
            "h_feats": h,
            "gy_fac": gy,
            "gx_fac": gx,
            "inp_pp": ipp,
            "inp_cn": icn,
            "offsets": offsets,
        })
    return in_maps


def _assemble(results):
    out = np.empty((C, N), np.float32)
    for r in range(NCORES):
        blk = results[r]["out_loc"].reshape(BLK, NMID // BLK, C)
        out[:, NMID * r:NMID * (r + 1)] = (
            blk.transpose(2, 1, 0).reshape(C, NMID))
    return out.reshape(1, C, H, W)


def _get_nc():
    global _CACHED_NC
    if _CACHED_NC is None:
        _CACHED_NC = _build_module()
    return _CACHED_NC


def run(input_tensor, reference_tensor, trace=False):
    from concourse.bass_utils import run_bass_kernel_spmd
    nc = _get_nc()
    in_maps = _host_inputs(input_tensor, reference_tensor)
    res = run_bass_kernel_spmd(nc, in_maps, core_ids=list(range(NCORES)),
                               trace=trace)
    return _assemble(res.results), res


def kernel(input_tensor, reference_tensor):
    out, _ = run(input_tensor, reference_tensor, trace=False)
    return out
